# revision 54
# baseline (speedup 1.0000x reference)
"""Trainium2 Bass kernel for nn_MultiHeadSelfAttention_88725434400988.

Self-contained: accepts FULL inputs, shards batch B=256 over 8 NeuronCores
(32 per core), runs one SPMD Bass program, gathers the FULL output.

v2 design notes (per-core; B_CORE=32, S=8, F=32, E=64, A=64, NH=2):
  - fp16 on the PE (1 cyc/row), fp32 PSUM.  End-to-end l2-rel err ~2e-3.
  - Attention row label: p = jh*128 + t*4 + sp  <->  orig = sp*64 + 2t + jh,
    (t = W col-tile, jh = 64-block parity, sp = s%4, head nh = s//4).
  - HAM discipline: the PE must never idle/stall >~3.4us or it re-throttles
    to 1.2 GHz and the whole attention phase runs at half clock (measured
    +40us).  So: v-proj is split around the two natural DMA bubbles (head
    DMA wait, qt partition-shift wait) with wide PSUM batching and copies
    alternating DVE/ACT so the PE stream never blocks on evacuation.
  - Q/K proj: lhsT = 128-col tiles of W, rhs = Hs^T, N=256 (roofline rate
    107ns/MM, 1024 MMs).  Wk first, Wq second, so kt's partition-shift
    DMAs overlap the Wq projection stream.
  - qt2/kt2 layout (128=nh*64+a, jh, b, 128=t*4+sp): nh on partition halves
    enables row-packed (tile_position) CONCURRENT zt matmul pairs and
    col-packed denominator/AV/residual pairs -> attention PE time halves.
    Psum gathers: 2 direct copies + 2 staged copies per group; one stage
    tile serves both cross-partition cases, drained by 2 bulk SBUF->SBUF
    DMAs per weight matrix (engines cannot cross partitions; DMA can).
  - Attention is transpose-free (Z^T), exp -> bf16 (fp32 range), softmax
    denominators land replicated via ones-lhsT PE matmul, single
    reciprocal_approx_fast, normalization fused into UT evacuation.
  - ACT is the phase-2 bottleneck (1 elem/cyc/lane): exp is ONE (128,1024)
    op per b; relu+bias epilogue moved to DVE (tensor_scalar add+max);
    output fp16.  Software-pipelined loop: PE does zt(b) while ACT exps
    z(b-1) and PE finishes denom/av(b-1) -> ACT-paced at ~1.25us/b.
"""
import numpy as np

B, S, F, E, A, NH = 256, 8, 32, 64, 64, 2
NCORES = 8
BC = B // NCORES            # 32 batches per core
ROWS = BC * S               # 256 projection rows
CD = F * E                  # 2048 contraction dim
ND = A * F * NH             # 4096 projection cols
KTILES = CD // 128          # 16
TTILES = ND // 128          # 32 column tiles per weight
NB = BC * NH                # 64 attention batches per core
WCHUNK = 2                  # weight tiles per DMA
GT = 2                      # projection tiles batched per psum/copy group

_NC_CACHE = None


def build_bass():
    import concourse.bacc as bacc
    import concourse.tile as tile
    from concourse import mybir

    f16 = mybir.dt.float16
    bf16 = mybir.dt.bfloat16
    f32 = mybir.dt.float32
    Exp = mybir.ActivationFunctionType.Exp
    Copy = mybir.ActivationFunctionType.Copy
    Add = mybir.AluOpType.add
    Max = mybir.AluOpType.max

    nc = bacc.Bacc("TRN2", target_bir_lowering=False, debug=False)

    # host-prepped layouts (see make_in_maps)
    hst_d = nc.dram_tensor("hst", [128, KTILES, ROWS], f16, kind="ExternalInput")
    hsv_d = nc.dram_tensor("hsv", [128, NB // 2, 128], f16, kind="ExternalInput")
    wq_d = nc.dram_tensor("wq", [128, TTILES, KTILES * 128], f16,
                          kind="ExternalInput")
    wk_d = nc.dram_tensor("wk", [128, TTILES, KTILES * 128], f16,
                          kind="ExternalInput")
    wv_d = nc.dram_tensor("wv", [E, 2 * A], f16, kind="ExternalInput")
    wres_d = nc.dram_tensor("wres", [2 * A, E], f16, kind="ExternalInput")
    bias_d = nc.dram_tensor("bias", [128, 1], f32, kind="ExternalInput")
    out_d = nc.dram_tensor("out", [128, BC // 4, 512], f16,
                           kind="ExternalOutput")

    with tile.TileContext(nc) as tc:
        from contextlib import ExitStack
        with ExitStack() as ctx:
            singles = ctx.enter_context(tc.tile_pool(name="singles", bufs=1))

            # ---- persistent tiles ----
            ones_bf = singles.tile([128, A], bf16)
            nc.vector.memset(ones_bf, 1.0)

            hsT = singles.tile([128, KTILES, ROWS], f16)
            hsv = singles.tile([128, NB // 2, 128], f16)
            wv_sb = singles.tile([128, 2 * A], f16)
            wres_sb = singles.tile([128, 2, E], f16)
            bias_sb = singles.tile([128, 1], f32)

            # head DMA priority: gpsimd queue carries ONLY the projection
            # critical path (hsT + wk chunks, few big DMAs); the scalar
            # queue feeds the v-proj warm-up (wv + hsv) in parallel.
            # small v-proj inputs lead the gpsimd queue: issued before the
            # heavy weight streams begin, they complete fast instead of
            # being starved by packet round-robin against 8KB-packet DMAs.
            # hsT rides the scalar queue in parallel with the first wk
            # chunks on gpsimd (both needed for the first projection MM).
            nc.gpsimd.dma_start(wv_sb[0:64, :], wv_d[:])
            nc.gpsimd.dma_start(wv_sb[64:128, :], wv_d[:])
            nc.gpsimd.dma_start(hsv[:, 0:16, :], hsv_d[:, 0:16, :])
            nc.scalar.dma_start(hsT[:, :, :], hst_d[:])
            for half in range(2):
                for jh in range(2):
                    nc.scalar.dma_start(
                        wres_sb[half * 64:(half + 1) * 64, jh, :],
                        wres_d[jh * 64:(jh + 1) * 64, :])
            nc.scalar.dma_start(bias_sb[:, :], bias_d[:])
            nc.scalar.dma_start(hsv[:, 16:, :], hsv_d[:, 16:, :])

            qt2 = singles.tile([128, 2, BC, 128], f16)   # (nh,a) jh b (t,sp)
            kt2 = singles.tile([128, 2, BC, 128], f16)
            v_all = singles.tile([128, NB, 2, A], bf16)  # sigma' bn kk a
            ut = singles.tile([128, BC, 2, 128], f16)    # (nh,a) b jh (t,sp)

            vp_ctx = ExitStack()
            vp_pool = vp_ctx.enter_context(
                tc.tile_pool(name="vps", bufs=1, space="PSUM"))
            dum_pool = vp_ctx.enter_context(
                tc.tile_pool(name="dum", bufs=1, space="PSUM"))

            def warm_burst(n, rhs):
                # dead accumulation into a throwaway psum tile: keeps the
                # HAM activity window busy through DMA-bound waits at ZERO
                # cost (no readers, no evacuation, PE-FIFO-only deps).
                dm = dum_pool.tile([64, 256], f32, name="dum", tag="dum")
                for i in range(n):
                    nc.tensor.matmul(dm[:, 0:rhs.shape[-1]], lhsT=ones_bf,
                                     rhs=rhs, start=(i == 0),
                                     stop=(i == n - 1))

            def vgroup(g, dve_only=False):
                # bns 4g..4g+3; concurrent row-tile pairs must drain to
                # DIFFERENT psum banks (per-(partition,bank) single-port
                # SRAMs) -> one tile per pi.
                vps = [vp_pool.tile([128, 2, 128], f32, name=f"vps{pi}",
                                    tag=f"vps{pi}") for pi in range(2)]
                for j in range(2):
                    for pi in range(2):
                        nc.tensor.matmul(
                            vps[pi][:, j, :],
                            lhsT=hsv[pi * 64:(pi + 1) * 64, 2 * g + j, :],
                            rhs=wv_sb[pi * 64:(pi + 1) * 64, :],
                            start=True, stop=True,
                            tile_position=(pi * 64, 0))
                for pi in range(2):
                    src = vps[pi].rearrange("p q (k a) -> p q k a", a=A)
                    # bn = 4g + 2j + pi -> strided dest slice
                    dst = v_all[:, 4 * g + pi:4 * g + pi + 3:2, :, :]
                    if pi == 0 or dve_only:
                        nc.vector.tensor_copy(dst, src)
                    else:
                        nc.scalar.activation(dst, src, Copy)

            # ---- head: dummy burst warms the PE from preamble-end while
            # the first DMAs land; then 8 v-proj groups run on real data.
            warm_burst(80, ones_bf)
            for g in range(8):
                vgroup(g)

            # ---- Q/K projection + batched gathers (Wk first) ----
            with tc.tile_pool(name="wtile", bufs=5) as w_pool, \
                 tc.tile_pool(name="stage", bufs=2) as st_pool, \
                 tc.tile_pool(name="pp", bufs=2, space="PSUM") as pp_pool:
                # first wk chunks are small so the projection stream starts
                # as soon as ~1.5MB has landed; later chunks are 2MB for
                # ~85% (vs 75% at 1MB) DMA efficiency so the weight stream
                # stays ahead of the PE.
                first_chunks = [(0, 1), (1, 1), (2, 2)] + [
                    (t, 4) for t in range(4, TTILES, 4)]
                for w_d, dest, dma_eng, chunks in (
                        (wk_d, kt2, nc.gpsimd, first_chunks),
                        (wq_d, qt2, nc.sync,
                         [(t, 4) for t in range(0, TTILES, 4)])):
                    stage = st_pool.tile([128, BC, 128], f16,
                                         name="stage", tag="stage")
                    for ci, (t0, cw) in enumerate(chunks):
                        wt = w_pool.tile([128, cw, KTILES, 128], f16,
                                         name="wt", tag="wt")
                        dma_eng.dma_start(
                            wt[:, :, :, :],
                            w_d[:, t0:t0 + cw, :]
                            .rearrange("p t (kt c) -> p t kt c", c=128))
                        # v-proj groups 8-15 pad the first (just-in-time)
                        # wk chunks so the PE never idles into a HAM
                        # re-throttle while the weight stream ramps.
                        if w_d is wk_d and ci <= 3:
                            vgroup(8 + 2 * ci)
                            vgroup(9 + 2 * ci)
                        pp = pp_pool.tile([128, cw, ROWS], f32,
                                          name="pp", tag="pp")
                        for ti in range(cw):
                            for kt in range(KTILES):
                                nc.tensor.matmul(
                                    pp[:, ti, :],
                                    lhsT=wt[:, ti, kt, :],
                                    rhs=hsT[:, kt, :],
                                    start=(kt == 0),
                                    stop=(kt == KTILES - 1))
                        # psum: [jh*64+a, ti, (b, nh, sp)]
                        src = pp.rearrange(
                            "p ti (b n sp) -> p n b ti sp", n=NH, sp=4)
                        ys = slice(t0 * 4, (t0 + cw) * 4)
                        dv = dest[:, :, :, ys].rearrange(
                            "p j b (ti sp) -> p j b ti sp", sp=4)
                        sv = stage[:, :, ys].rearrange(
                            "p b (ti sp) -> p b ti sp", sp=4)
                        # direct: (jh0,nh0) and (jh1,nh1)
                        nc.vector.tensor_copy(
                            dv[0:64, 0, :, :, :], src[0:64, 0, :, :, :])
                        nc.vector.tensor_copy(
                            dv[64:128, 1, :, :, :],
                            src[64:128, 1, :, :, :])
                        # staged: (jh1,nh0) stays on parts 64:128,
                        #         (jh0,nh1) stays on parts 0:64
                        # stage copies on the (idle) scalar engine: DVE
                        # stays under the MM group time and the last
                        # stage write lands right after the last MM.
                        nc.scalar.activation(
                            sv[64:128, :, :, :], src[64:128, 0, :, :, :],
                            Copy)
                        nc.scalar.activation(
                            sv[0:64, :, :, :], src[0:64, 1, :, :, :],
                            Copy)
                    # partition shifts (contiguous per-partition runs) split
                    # by b-range on 2 queues: attention's zt(b=0) only waits
                    # for the tiny first chunk; the rest overlaps early
                    # attention.  kt's overlap the Wq projection stream.
                    for b0, b1 in ((0, 4), (4, 12), (12, 22), (22, BC)):
                        nc.gpsimd.dma_start(dest[0:64, 1, b0:b1, :],
                                            stage[64:128, b0:b1, :])
                        nc.sync.dma_start(dest[64:128, 0, b0:b1, :],
                                          stage[0:64, b0:b1, :])

            # dummy burst bridges the qt-shift wait (no evacuation -> no
            # cross-engine backlog ahead of exp(0)/recip(0)).
            warm_burst(16, hsT[:, 0, :])
            vp_ctx.close()

            # ---- attention (transpose-free, nh-packed, ACT-paced) ----
            with tc.tile_pool(name="zps", bufs=2, space="PSUM") as z_pool, \
                 tc.tile_pool(name="dav", bufs=2, space="PSUM") as da_pool, \
                 tc.tile_pool(name="rps", bufs=2, space="PSUM") as r_pool, \
                 tc.tile_pool(name="expz", bufs=3) as e_pool, \
                 tc.tile_pool(name="reps", bufs=2) as rp_pool, \
                 tc.tile_pool(name="fo", bufs=2) as f_pool:

                def issue_z(b):
                    zt4 = z_pool.tile([128, 2, 2, 256], f32,
                                      name="zt", tag="zt")
                    for h in range(2):
                        for nh in range(2):
                            nc.tensor.matmul(
                                zt4[:, nh, h, :],
                                lhsT=kt2[nh * 64:(nh + 1) * 64, h, b, :],
                                rhs=qt2[nh * 64:(nh + 1) * 64, :, b, :],
                                start=True, stop=True,
                                tile_position=(nh * 64, 0))
                    ez = e_pool.tile([128, 2, 2, 256], bf16,
                                     name="ez", tag="ez")
                    if b == BC - 1:
                        # split the last exp so the drain chain's first
                        # denominator MMs start after the first half
                        for h in range(2):
                            nc.scalar.activation(
                                ez[:, :, h, :], zt4[:, :, h, :], Exp)
                    else:
                        nc.scalar.activation(
                            ez.rearrange("p n h y -> p (n h y)"),
                            zt4.rearrange("p n h y -> p (n h y)"), Exp)
                    return ez

                def issue_tail(b, ez):
                    dpr = da_pool.tile([128, 256], f32, name="dpr", tag="dav")
                    for nh in range(2):
                        for h in range(2):
                            nc.tensor.matmul(
                                dpr[nh * 64:(nh + 1) * 64, :],
                                lhsT=ones_bf[:, :],
                                rhs=ez[:, nh, h, :],
                                start=(h == 0), stop=(h == 1),
                                tile_position=(0, nh * 64))
                    av = da_pool.tile([128, 256], f32, name="av", tag="dav")
                    for nh in range(2):
                        for kk in range(2):
                            nc.tensor.matmul(
                                av[nh * 64:(nh + 1) * 64, :],
                                lhsT=v_all[:, b * NH + nh, kk, :],
                                rhs=ez[:, nh, kk, :],
                                start=(kk == 0), stop=(kk == 1),
                                tile_position=(0, nh * 64))
                    rep = rp_pool.tile([128, 256], f32, name="rep", tag="rep")
                    nc.vector.reciprocal_approx_fast(rep[:, :], dpr[:, :])
                    nc.vector.tensor_mul(
                        ut[:, b, :, :].rearrange("p j y -> p (j y)"),
                        av[:, :], rep[:, :])
                    # output col position (1-nh)*64 avoids the broken
                    # (64,64) array quadrant; host unpack swaps halves.
                    # Last group runs per-b so the final output DMA issues
                    # as early as possible (shrinks the serial tail).
                    if b >= BC - 4:
                        b4 = b % 4
                        rp_ = r_pool.tile([128, 512], f32, name="rp",
                                          tag="rp")
                        rp = rp_[:, 0:128]
                        for nh in range(2):
                            for jh in range(2):
                                nc.tensor.matmul(
                                    rp[(1 - nh) * 64:(2 - nh) * 64, :],
                                    lhsT=wres_sb[nh * 64:(nh + 1) * 64, jh, :],
                                    rhs=ut[nh * 64:(nh + 1) * 64,
                                           b:b + 1, jh, :],
                                    start=(jh == 0), stop=(jh == 1),
                                    tile_position=(nh * 64, (1 - nh) * 64))
                        fo = f_pool.tile([128, 128], f16, name="fol",
                                         tag="fol")
                        nc.vector.tensor_scalar(
                            fo[:, :], rp[:, :], bias_sb[:, :], 0.0, Add, Max)
                        nc.sync.dma_start(
                            out_d[:, BC // 4 - 1, b4 * 128:(b4 + 1) * 128],
                            fo[:, :])
                    elif b % 4 == 3:
                        bg = b // 4
                        rp = r_pool.tile([128, 512], f32, name="rp", tag="rp")
                        for nh in range(2):
                            for jh in range(2):
                                nc.tensor.matmul(
                                    rp[(1 - nh) * 64:(2 - nh) * 64, :],
                                    lhsT=wres_sb[nh * 64:(nh + 1) * 64, jh, :],
                                    rhs=ut[nh * 64:(nh + 1) * 64,
                                           bg * 4:(bg + 1) * 4, jh, :],
                                    start=(jh == 0), stop=(jh == 1),
                                    tile_position=(nh * 64, (1 - nh) * 64))
                        fo = f_pool.tile([128, 512], f16, name="fo", tag="fo")
                        nc.vector.tensor_scalar(
                            fo[:, :], rp[:, :], bias_sb[:, :], 0.0, Add, Max)
                        nc.sync.dma_start(out_d[:, bg, :], fo[:, :])

                ez_prev = None
                for b in range(BC):
                    ez_cur = issue_z(b)
                    if ez_prev is not None:
                        issue_tail(b - 1, ez_prev)
                    ez_prev = ez_cur
                issue_tail(BC - 1, ez_prev)
    nc.compile()
    return nc


def _get_nc():
    global _NC_CACHE
    if _NC_CACHE is None:
        _NC_CACHE = build_bass()
    return _NC_CACHE


def _prep_weight(W):
    # (CD, ND) -> (128, TTILES, KTILES*128): [p, t, kt*128+j] = W[kt*128+p, t*128+j]
    return np.ascontiguousarray(
        W.astype(np.float16).reshape(KTILES, 128, TTILES, 128)
        .transpose(1, 2, 0, 3).reshape(128, TTILES, KTILES * 128))


def make_in_maps(Hs, Wq, Wk, Wv, Wres_w, Wres_b):
    wq16 = _prep_weight(Wq)
    wk16 = _prep_weight(Wk)
    wv16 = Wv.astype(np.float16)
    wres16 = Wres_w.astype(np.float16)
    bias = np.tile(Wres_b.astype(np.float32).reshape(E, 1), (2, 1))
    hs16 = Hs.astype(np.float16)
    maps = []
    for c in range(NCORES):
        sh = hs16[c * BC:(c + 1) * BC]                      # (BC, S, CD)
        hs2d = sh.reshape(ROWS, CD)
        hst = np.ascontiguousarray(
            hs2d.reshape(ROWS, KTILES, 128).transpose(2, 1, 0))
        # v rows in sigma' order (t*4+sp):
        # hsv[pi*64+e, q, t*4+sp] = Hs[b, nh*4+sp, t, e]; bn = 2q+pi = b*NH+nh
        arr = sh.reshape(NB, 4, F, E).transpose(0, 2, 1, 3).reshape(NB, 128, E)
        hsv = np.ascontiguousarray(
            arr.reshape(NB // 2, 2, 128, E).transpose(1, 3, 0, 2)
            .reshape(128, NB // 2, 128))
        maps.append({
            "hst": hst, "hsv": hsv,
            "wq": wq16, "wk": wk16, "wv": wv16, "wres": wres16, "bias": bias,
        })
    return maps


def _unpack_out(o):
    # o: (128, 8, 512) = ((1-nh)*64+e, bg, (b4, t*4+sp)) -> (BC, S, F*E)
    o = o.reshape(NH, E, BC // 4, 4, F, 4)[::-1]
    return np.ascontiguousarray(
        o.transpose(2, 3, 0, 5, 4, 1)).reshape(BC, S, F * E)


def kernel(Hs, Wq, Wk, Wv, Wres_w, Wres_b):
    from concourse.bass_utils import run_bass_kernel_spmd
    nc = _get_nc()
    in_maps = make_in_maps(Hs, Wq, Wk, Wv, Wres_w, Wres_b)
    res = run_bass_kernel_spmd(nc, in_maps, list(range(NCORES)))
    out = np.concatenate(
        [_unpack_out(np.asarray(res.results[c]["out"]))
         for c in range(NCORES)], axis=0)
    return out.astype(np.float32)


if __name__ == "__main__":
    nc = build_bass()
    print("built OK; instructions:",
          sum(len(bb.instructions) for fn in nc.m.functions
              for bb in fn.blocks))


# revision 57
# speedup vs baseline: 1.1857x; 1.1857x over previous
"""Trainium2 Bass kernel for nn_MultiHeadSelfAttention_88725434400988.

Self-contained: accepts FULL inputs, shards batch B=256 over 8 NeuronCores
(32 per core), runs one SPMD Bass program, gathers the FULL output.

v2 design notes (per-core; B_CORE=32, S=8, F=32, E=64, A=64, NH=2):
  - fp16 on the PE (1 cyc/row), fp32 PSUM.  End-to-end l2-rel err ~2e-3.
  - Attention row label: p = jh*128 + t*4 + sp  <->  orig = sp*64 + 2t + jh,
    (t = W col-tile, jh = 64-block parity, sp = s%4, head nh = s//4).
  - HAM discipline: the PE must never idle/stall >~3.4us or it re-throttles
    to 1.2 GHz and the whole attention phase runs at half clock (measured
    +40us).  So: v-proj is split around the two natural DMA bubbles (head
    DMA wait, qt partition-shift wait) with wide PSUM batching and copies
    alternating DVE/ACT so the PE stream never blocks on evacuation.
  - Q/K proj: lhsT = 128-col tiles of W, rhs = Hs^T, N=256 (roofline rate
    107ns/MM, 1024 MMs).  Wk first, Wq second, so kt's partition-shift
    DMAs overlap the Wq projection stream.
  - qt2/kt2 layout (128=nh*64+a, jh, b, 128=t*4+sp): nh on partition halves
    enables row-packed (tile_position) CONCURRENT zt matmul pairs and
    col-packed denominator/AV/residual pairs -> attention PE time halves.
    Psum gathers: 2 direct copies + 2 staged copies per group; one stage
    tile serves both cross-partition cases, drained by 2 bulk SBUF->SBUF
    DMAs per weight matrix (engines cannot cross partitions; DMA can).
  - Attention is transpose-free (Z^T), exp -> bf16 (fp32 range), softmax
    denominators land replicated via ones-lhsT PE matmul, single
    reciprocal_approx_fast, normalization fused into UT evacuation.
  - ACT is the phase-2 bottleneck (1 elem/cyc/lane): exp is ONE (128,1024)
    op per b; relu+bias epilogue moved to DVE (tensor_scalar add+max);
    output fp16.  Software-pipelined loop: PE does zt(b) while ACT exps
    z(b-1) and PE finishes denom/av(b-1) -> ACT-paced at ~1.25us/b.
"""
import numpy as np

B, S, F, E, A, NH = 256, 8, 32, 64, 64, 2
NCORES = 8
BC = B // NCORES            # 32 batches per core
ROWS = BC * S               # 256 projection rows
CD = F * E                  # 2048 contraction dim
ND = A * F * NH             # 4096 projection cols
KTILES = CD // 128          # 16
TTILES = ND // 128          # 32 column tiles per weight
NB = BC * NH                # 64 attention batches per core
WCHUNK = 2                  # weight tiles per DMA
GT = 2                      # projection tiles batched per psum/copy group

_NC_CACHE = None


def build_bass():
    import concourse.bacc as bacc
    import concourse.tile as tile
    from concourse import mybir

    f16 = mybir.dt.float16
    bf16 = mybir.dt.bfloat16
    f32 = mybir.dt.float32
    Exp = mybir.ActivationFunctionType.Exp
    Copy = mybir.ActivationFunctionType.Copy
    Add = mybir.AluOpType.add
    Max = mybir.AluOpType.max

    nc = bacc.Bacc("TRN2", target_bir_lowering=False, debug=False)

    # host-prepped layouts (see make_in_maps)
    hst_d = nc.dram_tensor("hst", [128, KTILES, ROWS], f16, kind="ExternalInput")
    hsv_d = nc.dram_tensor("hsv", [128, NB // 2, 128], f16, kind="ExternalInput")
    wq_d = nc.dram_tensor("wq", [128, TTILES, KTILES * 128], f16,
                          kind="ExternalInput")
    wk_d = nc.dram_tensor("wk", [128, TTILES, KTILES * 128], f16,
                          kind="ExternalInput")
    wv_d = nc.dram_tensor("wv", [E, 2 * A], f16, kind="ExternalInput")
    wres_d = nc.dram_tensor("wres", [2 * A, E], f16, kind="ExternalInput")
    bias_d = nc.dram_tensor("bias", [128, 1], f32, kind="ExternalInput")
    out_d = nc.dram_tensor("out", [128, BC // 4, 512], f16,
                           kind="ExternalOutput")

    with tile.TileContext(nc) as tc:
        from contextlib import ExitStack
        with ExitStack() as ctx:
            singles = ctx.enter_context(tc.tile_pool(name="singles", bufs=1))

            # ---- persistent tiles ----
            ones_bf = singles.tile([128, A], bf16)
            nc.vector.memset(ones_bf, 1.0)

            hsT = singles.tile([128, KTILES, ROWS], f16)
            hsv = singles.tile([128, NB // 2, 128], f16)
            wv_sb = singles.tile([128, 2 * A], f16)
            wres_sb = singles.tile([128, 2, E], f16)
            bias_sb = singles.tile([128, 1], f32)

            # head DMA priority: gpsimd queue carries ONLY the projection
            # critical path (hsT + wk chunks, few big DMAs); the scalar
            # queue feeds the v-proj warm-up (wv + hsv) in parallel.
            # small v-proj inputs lead the gpsimd queue: issued before the
            # heavy weight streams begin, they complete fast instead of
            # being starved by packet round-robin against 8KB-packet DMAs.
            # hsT rides the scalar queue in parallel with the first wk
            # chunks on gpsimd (both needed for the first projection MM).
            nc.gpsimd.dma_start(wv_sb[0:64, :], wv_d[:])
            nc.gpsimd.dma_start(wv_sb[64:128, :], wv_d[:])
            nc.gpsimd.dma_start(hsv[:, 0:16, :], hsv_d[:, 0:16, :])
            nc.scalar.dma_start(hsT[:, :, :], hst_d[:])
            for half in range(2):
                for jh in range(2):
                    nc.scalar.dma_start(
                        wres_sb[half * 64:(half + 1) * 64, jh, :],
                        wres_d[jh * 64:(jh + 1) * 64, :])
            nc.scalar.dma_start(bias_sb[:, :], bias_d[:])
            nc.scalar.dma_start(hsv[:, 16:, :], hsv_d[:, 16:, :])

            qt2 = singles.tile([128, 2, BC, 128], f16)   # (nh,a) jh b (t,sp)
            kt2 = singles.tile([128, 2, BC, 128], f16)
            v_all = singles.tile([128, NB, 2, A], bf16)  # sigma' bn kk a
            ut = singles.tile([128, BC, 2, 128], f16)    # (nh,a) b jh (t,sp)

            vp_ctx = ExitStack()
            vp_pool = vp_ctx.enter_context(
                tc.tile_pool(name="vps", bufs=1, space="PSUM"))
            dum_pool = vp_ctx.enter_context(
                tc.tile_pool(name="dum", bufs=1, space="PSUM"))

            def warm_burst(n, rhs):
                # dead accumulation into a throwaway psum tile: keeps the
                # HAM activity window busy through DMA-bound waits at ZERO
                # cost (no readers, no evacuation, PE-FIFO-only deps).
                dm = dum_pool.tile([64, 256], f32, name="dum", tag="dum")
                for i in range(n):
                    nc.tensor.matmul(dm[:, 0:rhs.shape[-1]], lhsT=ones_bf,
                                     rhs=rhs, start=(i == 0),
                                     stop=(i == n - 1))

            def vgroup(g, dve_only=False):
                # bns 4g..4g+3; concurrent row-tile pairs must drain to
                # DIFFERENT psum banks (per-(partition,bank) single-port
                # SRAMs) -> one tile per pi.
                vps = [vp_pool.tile([128, 2, 128], f32, name=f"vps{pi}",
                                    tag=f"vps{pi}") for pi in range(2)]
                for j in range(2):
                    for pi in range(2):
                        nc.tensor.matmul(
                            vps[pi][:, j, :],
                            lhsT=hsv[pi * 64:(pi + 1) * 64, 2 * g + j, :],
                            rhs=wv_sb[pi * 64:(pi + 1) * 64, :],
                            start=True, stop=True,
                            tile_position=(pi * 64, 0))
                for pi in range(2):
                    src = vps[pi].rearrange("p q (k a) -> p q k a", a=A)
                    # bn = 4g + 2j + pi -> strided dest slice
                    dst = v_all[:, 4 * g + pi:4 * g + pi + 3:2, :, :]
                    if pi == 0 or dve_only:
                        nc.vector.tensor_copy(dst, src)
                    else:
                        nc.scalar.activation(dst, src, Copy)

            # ---- head: dummy burst warms the PE from preamble-end while
            # the first DMAs land; then 8 v-proj groups run on real data.
            warm_burst(120, ones_bf)
            for g in range(8):
                vgroup(g)

            # ---- Q/K projection + batched gathers (Wk first) ----
            with tc.tile_pool(name="wtile", bufs=5) as w_pool, \
                 tc.tile_pool(name="stage", bufs=2) as st_pool, \
                 tc.tile_pool(name="pp", bufs=2, space="PSUM") as pp_pool:
                # first wk chunks are small so the projection stream starts
                # as soon as ~1.5MB has landed; later chunks are 2MB for
                # ~85% (vs 75% at 1MB) DMA efficiency so the weight stream
                # stays ahead of the PE.
                first_chunks = [(0, 1), (1, 1), (2, 2)] + [
                    (t, 4) for t in range(4, TTILES, 4)]
                for w_d, dest, dma_eng, chunks in (
                        (wk_d, kt2, nc.gpsimd, first_chunks),
                        (wq_d, qt2, nc.sync,
                         [(t, 4) for t in range(0, TTILES, 4)])):
                    stage = st_pool.tile([128, BC, 128], f16,
                                         name="stage", tag="stage")
                    for ci, (t0, cw) in enumerate(chunks):
                        wt = w_pool.tile([128, cw, KTILES, 128], f16,
                                         name="wt", tag="wt")
                        dma_eng.dma_start(
                            wt[:, :, :, :],
                            w_d[:, t0:t0 + cw, :]
                            .rearrange("p t (kt c) -> p t kt c", c=128))
                        # v-proj groups 8-15 pad the first (just-in-time)
                        # wk chunks so the PE never idles into a HAM
                        # re-throttle while the weight stream ramps.
                        if w_d is wk_d and ci <= 3:
                            vgroup(8 + 2 * ci)
                            vgroup(9 + 2 * ci)
                        pp = pp_pool.tile([128, cw, ROWS], f32,
                                          name="pp", tag="pp")
                        for ti in range(cw):
                            for kt in range(KTILES):
                                nc.tensor.matmul(
                                    pp[:, ti, :],
                                    lhsT=wt[:, ti, kt, :],
                                    rhs=hsT[:, kt, :],
                                    start=(kt == 0),
                                    stop=(kt == KTILES - 1))
                        # psum: [jh*64+a, ti, (b, nh, sp)]
                        src = pp.rearrange(
                            "p ti (b n sp) -> p n b ti sp", n=NH, sp=4)
                        ys = slice(t0 * 4, (t0 + cw) * 4)
                        dv = dest[:, :, :, ys].rearrange(
                            "p j b (ti sp) -> p j b ti sp", sp=4)
                        sv = stage[:, :, ys].rearrange(
                            "p b (ti sp) -> p b ti sp", sp=4)
                        # direct: (jh0,nh0) and (jh1,nh1)
                        nc.vector.tensor_copy(
                            dv[0:64, 0, :, :, :], src[0:64, 0, :, :, :])
                        nc.vector.tensor_copy(
                            dv[64:128, 1, :, :, :],
                            src[64:128, 1, :, :, :])
                        # staged: (jh1,nh0) stays on parts 64:128,
                        #         (jh0,nh1) stays on parts 0:64
                        # stage copies on the (idle) scalar engine: DVE
                        # stays under the MM group time and the last
                        # stage write lands right after the last MM.
                        # In the LAST wq chunk, the b0:4 slice goes first
                        # so the b0:4 partition-shift (which gates zt(b=0))
                        # issues as early as possible.
                        if w_d is wq_d and ci == len(chunks) - 1:
                            nc.scalar.activation(
                                sv[64:128, 0:4, :, :],
                                src[64:128, 0, 0:4, :, :], Copy)
                            nc.scalar.activation(
                                sv[0:64, 0:4, :, :],
                                src[0:64, 1, 0:4, :, :], Copy)
                            nc.scalar.activation(
                                sv[64:128, 4:, :, :],
                                src[64:128, 0, 4:, :, :], Copy)
                            nc.scalar.activation(
                                sv[0:64, 4:, :, :],
                                src[0:64, 1, 4:, :, :], Copy)
                        else:
                            nc.scalar.activation(
                                sv[64:128, :, :, :], src[64:128, 0, :, :, :],
                                Copy)
                            nc.scalar.activation(
                                sv[0:64, :, :, :], src[0:64, 1, :, :, :],
                                Copy)
                    # partition shifts (contiguous per-partition runs) split
                    # by b-range on 2 queues: attention's zt(b=0) only waits
                    # for the tiny first chunk; the rest overlaps early
                    # attention.  kt's overlap the Wq projection stream.
                    for b0, b1 in ((0, 4), (4, 12), (12, 22), (22, BC)):
                        nc.gpsimd.dma_start(dest[0:64, 1, b0:b1, :],
                                            stage[64:128, b0:b1, :])
                        nc.sync.dma_start(dest[64:128, 0, b0:b1, :],
                                          stage[0:64, b0:b1, :])

            # dummy burst bridges the qt-shift wait (no evacuation -> no
            # cross-engine backlog ahead of exp(0)/recip(0)).
            warm_burst(24, hsT[:, 0, :])
            vp_ctx.close()

            # ---- attention (transpose-free, nh-packed, ACT-paced) ----
            with tc.tile_pool(name="zps", bufs=2, space="PSUM") as z_pool, \
                 tc.tile_pool(name="dav", bufs=2, space="PSUM") as da_pool, \
                 tc.tile_pool(name="rps", bufs=2, space="PSUM") as r_pool, \
                 tc.tile_pool(name="expz", bufs=3) as e_pool, \
                 tc.tile_pool(name="reps", bufs=2) as rp_pool, \
                 tc.tile_pool(name="fo", bufs=2) as f_pool:

                def issue_z(b):
                    zt4 = z_pool.tile([128, 2, 2, 256], f32,
                                      name="zt", tag="zt")
                    for h in range(2):
                        for nh in range(2):
                            nc.tensor.matmul(
                                zt4[:, nh, h, :],
                                lhsT=kt2[nh * 64:(nh + 1) * 64, h, b, :],
                                rhs=qt2[nh * 64:(nh + 1) * 64, :, b, :],
                                start=True, stop=True,
                                tile_position=(nh * 64, 0))
                    ez = e_pool.tile([128, 2, 2, 256], bf16,
                                     name="ez", tag="ez")
                    if b == BC - 1:
                        # split the last exp so the drain chain's first
                        # denominator MMs start after the first half
                        for h in range(2):
                            nc.scalar.activation(
                                ez[:, :, h, :], zt4[:, :, h, :], Exp)
                    else:
                        nc.scalar.activation(
                            ez.rearrange("p n h y -> p (n h y)"),
                            zt4.rearrange("p n h y -> p (n h y)"), Exp)
                    return ez

                def issue_tail(b, ez):
                    dpr = da_pool.tile([128, 256], f32, name="dpr", tag="dav")
                    for nh in range(2):
                        for h in range(2):
                            nc.tensor.matmul(
                                dpr[nh * 64:(nh + 1) * 64, :],
                                lhsT=ones_bf[:, :],
                                rhs=ez[:, nh, h, :],
                                start=(h == 0), stop=(h == 1),
                                tile_position=(0, nh * 64))
                    av = da_pool.tile([128, 256], f32, name="av", tag="dav")
                    for nh in range(2):
                        for kk in range(2):
                            nc.tensor.matmul(
                                av[nh * 64:(nh + 1) * 64, :],
                                lhsT=v_all[:, b * NH + nh, kk, :],
                                rhs=ez[:, nh, kk, :],
                                start=(kk == 0), stop=(kk == 1),
                                tile_position=(0, nh * 64))
                    rep = rp_pool.tile([128, 256], f32, name="rep", tag="rep")
                    nc.vector.reciprocal_approx_fast(rep[:, :], dpr[:, :])
                    nc.vector.tensor_mul(
                        ut[:, b, :, :].rearrange("p j y -> p (j y)"),
                        av[:, :], rep[:, :])
                    # output col position (1-nh)*64 avoids the broken
                    # (64,64) array quadrant; host unpack swaps halves.
                    # Last group runs per-b so the final output DMA issues
                    # as early as possible (shrinks the serial tail).
                    if b >= BC - 4:
                        b4 = b % 4
                        rp_ = r_pool.tile([128, 512], f32, name="rp",
                                          tag="rp")
                        rp = rp_[:, 0:128]
                        for nh in range(2):
                            for jh in range(2):
                                nc.tensor.matmul(
                                    rp[(1 - nh) * 64:(2 - nh) * 64, :],
                                    lhsT=wres_sb[nh * 64:(nh + 1) * 64, jh, :],
                                    rhs=ut[nh * 64:(nh + 1) * 64,
                                           b:b + 1, jh, :],
                                    start=(jh == 0), stop=(jh == 1),
                                    tile_position=(nh * 64, (1 - nh) * 64))
                        fo = f_pool.tile([128, 128], f16, name="fol",
                                         tag="fol")
                        nc.vector.tensor_scalar(
                            fo[:, :], rp[:, :], bias_sb[:, :], 0.0, Add, Max)
                        nc.sync.dma_start(
                            out_d[:, BC // 4 - 1, b4 * 128:(b4 + 1) * 128],
                            fo[:, :])
                    elif b % 4 == 3:
                        bg = b // 4
                        rp = r_pool.tile([128, 512], f32, name="rp", tag="rp")
                        for nh in range(2):
                            for jh in range(2):
                                nc.tensor.matmul(
                                    rp[(1 - nh) * 64:(2 - nh) * 64, :],
                                    lhsT=wres_sb[nh * 64:(nh + 1) * 64, jh, :],
                                    rhs=ut[nh * 64:(nh + 1) * 64,
                                           bg * 4:(bg + 1) * 4, jh, :],
                                    start=(jh == 0), stop=(jh == 1),
                                    tile_position=(nh * 64, (1 - nh) * 64))
                        fo = f_pool.tile([128, 512], f16, name="fo", tag="fo")
                        nc.vector.tensor_scalar(
                            fo[:, :], rp[:, :], bias_sb[:, :], 0.0, Add, Max)
                        nc.sync.dma_start(out_d[:, bg, :], fo[:, :])

                ez_prev = None
                for b in range(BC):
                    ez_cur = issue_z(b)
                    if ez_prev is not None:
                        issue_tail(b - 1, ez_prev)
                    ez_prev = ez_cur
                issue_tail(BC - 1, ez_prev)
    nc.compile()
    return nc


def _get_nc():
    global _NC_CACHE
    if _NC_CACHE is None:
        _NC_CACHE = build_bass()
    return _NC_CACHE


def _prep_weight(W):
    # (CD, ND) -> (128, TTILES, KTILES*128): [p, t, kt*128+j] = W[kt*128+p, t*128+j]
    return np.ascontiguousarray(
        W.astype(np.float16).reshape(KTILES, 128, TTILES, 128)
        .transpose(1, 2, 0, 3).reshape(128, TTILES, KTILES * 128))


def make_in_maps(Hs, Wq, Wk, Wv, Wres_w, Wres_b):
    wq16 = _prep_weight(Wq)
    wk16 = _prep_weight(Wk)
    wv16 = Wv.astype(np.float16)
    wres16 = Wres_w.astype(np.float16)
    bias = np.tile(Wres_b.astype(np.float32).reshape(E, 1), (2, 1))
    hs16 = Hs.astype(np.float16)
    maps = []
    for c in range(NCORES):
        sh = hs16[c * BC:(c + 1) * BC]                      # (BC, S, CD)
        hs2d = sh.reshape(ROWS, CD)
        hst = np.ascontiguousarray(
            hs2d.reshape(ROWS, KTILES, 128).transpose(2, 1, 0))
        # v rows in sigma' order (t*4+sp):
        # hsv[pi*64+e, q, t*4+sp] = Hs[b, nh*4+sp, t, e]; bn = 2q+pi = b*NH+nh
        arr = sh.reshape(NB, 4, F, E).transpose(0, 2, 1, 3).reshape(NB, 128, E)
        hsv = np.ascontiguousarray(
            arr.reshape(NB // 2, 2, 128, E).transpose(1, 3, 0, 2)
            .reshape(128, NB // 2, 128))
        maps.append({
            "hst": hst, "hsv": hsv,
            "wq": wq16, "wk": wk16, "wv": wv16, "wres": wres16, "bias": bias,
        })
    return maps


def _unpack_out(o):
    # o: (128, 8, 512) = ((1-nh)*64+e, bg, (b4, t*4+sp)) -> (BC, S, F*E)
    o = o.reshape(NH, E, BC // 4, 4, F, 4)[::-1]
    return np.ascontiguousarray(
        o.transpose(2, 3, 0, 5, 4, 1)).reshape(BC, S, F * E)


def kernel(Hs, Wq, Wk, Wv, Wres_w, Wres_b):
    from concourse.bass_utils import run_bass_kernel_spmd
    nc = _get_nc()
    in_maps = make_in_maps(Hs, Wq, Wk, Wv, Wres_w, Wres_b)
    res = run_bass_kernel_spmd(nc, in_maps, list(range(NCORES)))
    out = np.concatenate(
        [_unpack_out(np.asarray(res.results[c]["out"]))
         for c in range(NCORES)], axis=0)
    return out.astype(np.float32)


if __name__ == "__main__":
    nc = build_bass()
    print("built OK; instructions:",
          sum(len(bb.instructions) for fn in nc.m.functions
              for bb in fn.blocks))
